# revision 10
# baseline (speedup 1.0000x reference)
"""Distributed RoPE causal attention for Trainium2 (8 NeuronCores).

Problem: B=2, L=2048, H=16 heads, D=64; y = Attn(x) with RoPE on q,k,
causal softmax, then output projection. fp32 I/O.

Sharding: each core owns 2 heads (tensor-parallel over the 16 heads) for
BOTH batches.  Per (chunk, batch) an 8-core AllToAll redistributes the
per-head attention outputs so that core c receives all 16 heads for the
64-wide l-slice [64c, 64c+64) of that chunk; the (replicated-weight)
output projection for each chunk then runs pipelined one chunk behind
the attention compute, so only the last small AllToAll is exposed.

RoPE trick: the q/k head dims are PERMUTED (python-side, in the weight
shards and cos/sin tables) so partition 2k holds dim k and 2k+1 holds
dim k+32.  rotate_half then becomes a single intra-quadrant
stream_shuffle (mask i^1) with the signs folded into the sin table.
Scores are invariant to the permutation (q and k share it).

Matmuls run in bf16 (fp32 PSUM accumulation); softmax plumbing fp32.
"""
import sys

sys.path.insert(0, "/opt/trn_rl_repo")

import numpy as np
import ml_dtypes

import concourse.bass as bass
import concourse.tile as tile
from concourse import bacc, mybir
from concourse import bass_utils

B, L, H, D = 2, 2048, 16, 64
HID = H * D
NC = 8
CH = 512          # lq chunk width
NCH = L // CH     # 4 chunks per batch
NT = L // 128     # 16 k-tiles of 128 per batch
F32 = mybir.dt.float32
BF16 = mybir.dt.bfloat16
AF = mybir.ActivationFunctionType
ALU = mybir.AluOpType

ROT_MASK = [i ^ 1 for i in range(32)]

_CACHE = {}


def build():
    nc = bacc.Bacc("TRN2", target_bir_lowering=False, debug=False, num_devices=NC)

    xT_e = nc.dram_tensor("xT", [B, HID, L], BF16, kind="ExternalInput")
    wq_e = nc.dram_tensor("wq", [HID, 128], BF16, kind="ExternalInput")
    wk_e = nc.dram_tensor("wk", [HID, 128], BF16, kind="ExternalInput")
    wv_e = nc.dram_tensor("wv", [HID, 128], BF16, kind="ExternalInput")
    wo_e = nc.dram_tensor("wo", [HID, HID], BF16, kind="ExternalInput")
    cos_e = nc.dram_tensor("cosp", [128, L], BF16, kind="ExternalInput")
    sin_e = nc.dram_tensor("sinpm", [128, L], BF16, kind="ExternalInput")
    tri_e = nc.dram_tensor("tri", [128, 128], BF16, kind="ExternalInput")
    out_e = nc.dram_tensor("out", [NCH, 128, HID], F32, kind="ExternalOutput")

    xT_r = xT_e.ap().rearrange("b (kt p) l -> b p kt l", p=128)

    with tile.TileContext(nc) as tc:
        with tc.tile_pool(name="const", bufs=1) as cpool, \
             tc.tile_pool(name="persist", bufs=1) as ppool, \
             tc.tile_pool(name="xin", bufs=4) as xpool, \
             tc.tile_pool(name="tmp", bufs=3) as tpool, \
             tc.tile_pool(name="ptp", bufs=8) as ptpool, \
             tc.tile_pool(name="osb", bufs=3) as opool, \
             tc.tile_pool(name="fin", bufs=2) as fpool, \
             tc.tile_pool(name="ps_proj", bufs=2, space="PSUM") as ps_proj, \
             tc.tile_pool(name="ps_s", bufs=2, space="PSUM") as ps_s, \
             tc.tile_pool(name="ps_o", bufs=1, space="PSUM") as ps_o, \
             tc.tile_pool(name="dram", bufs=1, space="DRAM") as dpool:

            wq_sb = cpool.tile([128, 8, 128], BF16)
            wk_sb = cpool.tile([128, 8, 128], BF16)
            wv_sb = cpool.tile([128, 8, 128], BF16)
            wo_sb = cpool.tile([128, 8, HID], BF16)
            nc.sync.dma_start(wq_sb[:], wq_e.ap().rearrange("(kt p) m -> p kt m", p=128))
            nc.sync.dma_start(wk_sb[:], wk_e.ap().rearrange("(kt p) m -> p kt m", p=128))
            nc.sync.dma_start(wv_sb[:], wv_e.ap().rearrange("(kt p) m -> p kt m", p=128))
            cos_sb = cpool.tile([128, L], BF16)
            sin_sb = cpool.tile([128, L], BF16)
            nc.sync.dma_start(cos_sb[:], cos_e[:, :])
            nc.sync.dma_start(sin_sb[:], sin_e[:, :])
            tri2_sb = cpool.tile([128, 2, 128], BF16)
            nc.sync.dma_start(tri2_sb[:, 0], tri_e[:, :])
            nc.sync.dma_start(tri2_sb[:, 1], tri_e[:, :])
            nc.sync.dma_start(
                wo_sb[:], wo_e.ap().rearrange("(kt p) m -> p kt m", p=128))

            # persistent per-core tensors
            qT_sb = ppool.tile([128, B, L], BF16)   # [64*hl+dperm, b, l]
            kT_sb = ppool.tile([128, B, L], BF16)
            # [lk%128, b, lk//128, 65*hl + (d|ones)]; ones column at e=64
            v_sb = ppool.tile([128, B, NT, 130], BF16)

            # AllToAll buffers, one per (chunk, batch):
            # [peer, 128 rows = 64h+d, 64-wide l slice]
            a2a_in = [[dpool.tile([NC, 128, 64], BF16, name=f"a2i{j}{b}",
                                  tag=f"a2i{j}{b}") for b in range(B)]
                      for j in range(NCH)]
            a2a_out = [[dpool.tile([NC, 128, 64], BF16, name=f"a2o{j}{b}",
                                   tag=f"a2o{j}{b}") for b in range(B)]
                       for j in range(NCH)]

            def emit_wo(jj):
                # output projection for chunk jj from the gathered pieces
                oF = fpool.tile([128, 8, B, 64], BF16, name="oF")
                for b in range(B):
                    for p in range(NC):
                        nc.gpsimd.dma_start(oF[:, p, b, :], a2a_out[jj][b][p])
                for nn in range(2):
                    py_ = ps_proj.tile([128, 512], F32, tag="proj", name="py")
                    for p in range(8):
                        nc.tensor.matmul(py_[:], oF[:, p],
                                         wo_sb[:, p, 512 * nn:512 * nn + 512],
                                         start=(p == 0), stop=(p == 7))
                    ysb = fpool.tile([128, 512], F32, name="ysb")
                    nc.scalar.activation(ysb[:], py_[:], AF.Copy)
                    nc.sync.dma_start(out_e[jj][:, 512 * nn:512 * nn + 512], ysb[:])

            for j in range(NCH):
                if j == NCH - 1:
                    # A2A(0,1)/A2A(1,1) have had 1-2 chunks to complete by now
                    emit_wo(0)
                    emit_wo(1)
                for b in range(B):
                    ls = j * CH
                    xc = xpool.tile([128, 8, CH], BF16, name="xc")
                    for kt in range(8):
                        # per-k-tile loads: the first projection matmul starts
                        # as soon as its slice lands (subtile deps)
                        nc.sync.dma_start(xc[:, kt], xT_r[b][:, kt, ls:ls + CH])

                    # ---- Q, K projections + RoPE (permuted-pair layout) ----
                    for w_sb, dst in ((wq_sb, qT_sb), (wk_sb, kT_sb)):
                        pp = ps_proj.tile([128, CH], F32, tag="proj", name="pp")
                        for kt in range(8):
                            nc.tensor.matmul(pp[:], w_sb[:, kt], xc[:, kt],
                                             start=(kt == 0), stop=(kt == 7))
                        t1 = tpool.tile([128, CH], BF16, name="t1")
                        nc.vector.tensor_mul(t1[:], pp[:], cos_sb[:, ls:ls + CH])
                        sh = tpool.tile([128, CH], F32, name="sh")
                        nc.vector.stream_shuffle(sh[:], pp[:], ROT_MASK)
                        t2 = tpool.tile([128, CH], BF16, name="t2")
                        nc.vector.tensor_mul(t2[:], sh[:], sin_sb[:, ls:ls + CH])
                        nc.vector.tensor_add(dst[:, b, ls:ls + CH], t1[:], t2[:])

                    # ---- V projection (+ ones column for denominators) ----
                    for tl in range(4):
                        t = 4 * j + tl
                        pv = ps_proj.tile([128, 128], F32, tag="proj", name="pv")
                        for kt in range(8):
                            nc.tensor.matmul(pv[:], xc[:, kt, 128 * tl:128 * tl + 128],
                                             wv_sb[:, kt], start=(kt == 0), stop=(kt == 7))
                        nc.vector.memset(v_sb[:, b, t, 64::65], 1.0)
                        vv = v_sb.rearrange("p b t (hl e) -> p b t hl e", e=65)
                        nc.vector.tensor_copy(
                            vv[:, b, t, :, 0:64],
                            pv.rearrange("p (hl d) -> p hl d", d=64),
                        )

                    # ---- attention for lq chunk j, both local heads ----
                    # po partitions: 0..63 = o dims, 64 = denominator (ones row)
                    po = ps_o.tile([65, 2, CH], F32, tag="o", name="po")
                    nt_j = 4 * j + 4
                    pts = [None] * nt_j
                    o0s = [0] * nt_j

                    def emit_av(t):
                        for hl in range(2):
                            nc.tensor.matmul(po[:, hl, o0s[t]:CH],
                                             v_sb[:, b, t, 65 * hl:65 * hl + 65],
                                             pts[t][:, hl, o0s[t]:CH],
                                             start=(t == 0), stop=(t == nt_j - 1),
                                             skip_group_check=True)

                    for t in range(nt_j):
                        tl = t - 4 * j
                        o0 = 128 * tl if tl > 0 else 0   # skip fully-masked cols
                        o0s[t] = o0
                        # both heads' scores in one 2-bank psum tile so exp
                        # and the diagonal mask run as single wider ops
                        pss = ps_s.tile([128, 2, CH], F32, tag="s", name="pss")
                        for hl in range(2):
                            hp = 64 * hl
                            nc.tensor.matmul(
                                pss[:, hl, o0:CH],
                                kT_sb[hp:hp + 64, b, 128 * t:128 * t + 128],
                                qT_sb[hp:hp + 64, b, ls + o0:ls + CH],
                                start=True, stop=True)
                        pt = ptpool.tile([128, 2, CH], BF16, name="pt")
                        nc.scalar.activation(pt[:, :, o0:CH], pss[:, :, o0:CH],
                                             AF.Exp, scale=0.125)
                        pts[t] = pt
                        if tl >= 0:
                            nc.vector.tensor_mul(pt[:, :, o0:o0 + 128],
                                                 pt[:, :, o0:o0 + 128], tri2_sb[:])
                        # software-pipeline the PE stream: scores(t) are queued
                        # ahead of AV(t-1) so the PE never head-of-line blocks
                        # on the exp of the current tile
                        if t > 0:
                            emit_av(t - 1)
                    emit_av(nt_j - 1)

                    # ---- epilogue: divide by denominator, ship to AllToAll ----
                    # NB: custom-DVE recip misreads PSUM at base partition 64,
                    # so stage the denominator rows through SBUF first.
                    dsb = tpool.tile([1, 2, CH], F32, name="dsb", bufs=2)
                    nc.vector.tensor_copy(dsb[:], po[64:65, :, :])
                    rec = tpool.tile([1, 2, CH], F32, name="rec", bufs=2)
                    nc.vector.reciprocal_approx_fast(rec[:], dsb[:])
                    rb = tpool.tile([64, 2, CH], F32, name="rb", bufs=2)
                    nc.gpsimd.partition_broadcast(rb[:], rec[:])
                    o_sb = opool.tile([64, 2, CH], BF16, name="o_sb")
                    nc.vector.tensor_mul(o_sb[:], po[0:64, :, :], rb[:])
                    # piece rows (h d), cols l  <-  [d, h, l-slice c]; per peer,
                    # issued from the Pool queue (cheap dispatch, feeds the
                    # collective trigger that follows on the same queue)
                    for c in range(NC):
                        nc.gpsimd.dma_start(
                            a2a_in[j][b][c].rearrange("(h d) l -> d h l", h=2),
                            o_sb[:, :, 64 * c:64 * c + 64])
                    nc.gpsimd.collective_compute(
                        "AllToAll", ALU.bypass,
                        replica_groups=[list(range(NC))],
                        ins=[a2a_in[j][b][:].opt()],
                        outs=[a2a_out[j][b][:].opt()],
                    )

            emit_wo(NCH - 2)
            emit_wo(NCH - 1)

    nc.compile()
    return nc


def _perm():
    # partition 2k <- dim k, partition 2k+1 <- dim k+32 (per 64-dim head)
    p = np.empty(64, dtype=np.int64)
    k = np.arange(32)
    p[2 * k] = k
    p[2 * k + 1] = k + 32
    return p


def _shards(x, Wq, Wk, Wv, Wo, cos, sin):
    bf = ml_dtypes.bfloat16
    xT = np.ascontiguousarray(x.transpose(0, 2, 1)).astype(bf)          # (B, HID, L)
    woT = np.ascontiguousarray(Wo.T).astype(bf)                          # (HID, HID)
    perm = _perm()

    cosT = cos.T.astype(np.float32)                                      # (D, L)
    sinT = sin.T.astype(np.float32)
    # permuted tables: row 2k -> (cos[k], -sin[k]); row 2k+1 -> (cos[k+32], sin[k+32])
    cosP = cosT[perm]                                                    # (64, L)
    sgn = np.where(np.arange(64) % 2 == 0, -1.0, 1.0)[:, None].astype(np.float32)
    sinP = sinT[perm] * sgn                                              # (64, L)
    cos2 = np.concatenate([cosP, cosP], axis=0).astype(bf)               # (128, L)
    sin2 = np.ascontiguousarray(np.concatenate([sinP, sinP], axis=0)).astype(bf)
    # lower-triangular 128x128 mask for the diagonal band
    p = np.arange(128)[:, None]
    f = np.arange(128)[None, :]
    tri = (p <= f).astype(np.float32).astype(bf)                         # (128, 128)

    hperm = np.concatenate([perm, perm + 64])                            # (128,)

    in_maps = []
    for c in range(NC):
        rows = slice(2 * c * 64, (2 * c + 2) * 64)
        wqT = np.ascontiguousarray(Wq[rows].T[:, hperm]).astype(bf)
        wkT = np.ascontiguousarray(Wk[rows].T[:, hperm]).astype(bf)
        in_maps.append({
            "xT": xT,
            "wq": wqT,
            "wk": wkT,
            "wv": np.ascontiguousarray(Wv[rows].T).astype(bf),
            "wo": woT,
            "cosp": cos2,
            "sinpm": sin2,
            "tri": tri,
        })
    return in_maps


def kernel(x, Wq, Wk, Wv, Wo, cos, sin, trace=False):
    x = np.asarray(x, dtype=np.float32)
    Wq = np.asarray(Wq, dtype=np.float32)
    Wk = np.asarray(Wk, dtype=np.float32)
    Wv = np.asarray(Wv, dtype=np.float32)
    Wo = np.asarray(Wo, dtype=np.float32)
    cos = np.asarray(cos, dtype=np.float32)
    sin = np.asarray(sin, dtype=np.float32)

    if "nc" not in _CACHE:
        _CACHE["nc"] = build()
    nc = _CACHE["nc"]

    in_maps = _shards(x, Wq, Wk, Wv, Wo, cos, sin)
    kw = {}
    if trace and _CACHE.get("trace_all_cores"):
        kw["trace_cores"] = list(range(NC))
    res = bass_utils.run_bass_kernel_spmd(
        nc, in_maps, core_ids=list(range(NC)), trace=trace, **kw)
    _CACHE["last_result"] = res

    y = np.empty((B, L, HID), dtype=np.float32)
    for c in range(NC):
        o = res.results[c]["out"]                     # (NCH, 128, HID)
        for j in range(NCH):
            for b in range(B):
                y[b, CH * j + 64 * c: CH * j + 64 * c + 64, :] = \
                    o[j, 64 * b:64 * b + 64, :]
    return y


if __name__ == "__main__":
    rng = np.random.default_rng(0)
    sc = 1.0 / np.sqrt(HID)
    inputs = {
        "x": rng.standard_normal((B, L, HID), dtype=np.float32),
        "Wq": rng.standard_normal((HID, HID), dtype=np.float32) * sc,
        "Wk": rng.standard_normal((HID, HID), dtype=np.float32) * sc,
        "Wv": rng.standard_normal((HID, HID), dtype=np.float32) * sc,
        "Wo": rng.standard_normal((HID, HID), dtype=np.float32) * sc,
        "cos": rng.random((L, D), dtype=np.float32),
        "sin": rng.random((L, D), dtype=np.float32),
    }
    y = kernel(**inputs)
    print("ran:", y.shape, y.dtype)


# revision 11
# speedup vs baseline: 1.1068x; 1.1068x over previous
"""Distributed RoPE causal attention for Trainium2 (8 NeuronCores).

Problem: B=2, L=2048, H=16 heads, D=64; y = Attn(x) with RoPE on q,k,
causal softmax, then output projection. fp32 I/O.

Sharding: each core owns 2 heads (tensor-parallel over the 16 heads) for
BOTH batches.  Per (chunk, batch) an 8-core AllToAll redistributes the
per-head attention outputs so that core c receives all 16 heads for the
64-wide l-slice [64c, 64c+64) of that chunk; the (replicated-weight)
output projection for each chunk is interleaved into later attention
loops, so only the last small AllToAll is exposed.

The attention t-loop is ACT-(exp)-bound; the next chunk's QKV
projection matmuls and the pending output projections are emitted as
filler steps INSIDE the t-loop so the PE never idles (and stays at max
p-state).

RoPE trick: the q/k head dims are PERMUTED (python-side, in the weight
shards and cos/sin tables) so partition 2k holds dim k and 2k+1 holds
dim k+32.  rotate_half then becomes a single intra-quadrant
stream_shuffle (mask i^1) with the signs folded into the sin table.
Scores are invariant to the permutation (q and k share it).

Matmuls run in bf16 (fp32 PSUM accumulation); softmax plumbing fp32.
"""
import math
import sys

sys.path.insert(0, "/opt/trn_rl_repo")

import numpy as np
import ml_dtypes

import concourse.bass as bass
import concourse.tile as tile
from concourse import bacc, mybir
from concourse import bass_utils

B, L, H, D = 2, 2048, 16, 64
HID = H * D
NC = 8
CH = 512          # lq chunk width
NCH = L // CH     # 4 chunks per batch
NT = L // 128     # 16 k-tiles of 128 per batch
F32 = mybir.dt.float32
BF16 = mybir.dt.bfloat16
AF = mybir.ActivationFunctionType
ALU = mybir.AluOpType

ROT_MASK = [i ^ 1 for i in range(32)]

_CACHE = {}


class Feeder:
    """Queue of emission closures drained evenly across t-loop iterations."""

    def __init__(self):
        self.q = []

    def add(self, steps):
        self.q.extend(steps)

    def take(self, remaining_iters):
        if not self.q:
            return []
        n = math.ceil(len(self.q) / max(remaining_iters, 1))
        out, self.q = self.q[:n], self.q[n:]
        return out

    def drain(self):
        out, self.q = self.q, []
        return out


def build():
    nc = bacc.Bacc("TRN2", target_bir_lowering=False, debug=False, num_devices=NC)

    xT_e = nc.dram_tensor("xT", [B, HID, L], BF16, kind="ExternalInput")
    wq_e = nc.dram_tensor("wq", [HID, 128], BF16, kind="ExternalInput")
    wk_e = nc.dram_tensor("wk", [HID, 128], BF16, kind="ExternalInput")
    wv_e = nc.dram_tensor("wv", [HID, 128], BF16, kind="ExternalInput")
    wo_e = nc.dram_tensor("wo", [HID, HID], BF16, kind="ExternalInput")
    cos_e = nc.dram_tensor("cosp", [128, L], BF16, kind="ExternalInput")
    sin_e = nc.dram_tensor("sinpm", [128, L], BF16, kind="ExternalInput")
    tri_e = nc.dram_tensor("tri", [128, 128], BF16, kind="ExternalInput")
    out_e = nc.dram_tensor("out", [NCH, 128, HID], F32, kind="ExternalOutput")

    xT_r = xT_e.ap().rearrange("b (kt p) l -> b p kt l", p=128)

    with tile.TileContext(nc) as tc:
        with tc.tile_pool(name="const", bufs=1) as cpool, \
             tc.tile_pool(name="persist", bufs=1) as ppool, \
             tc.tile_pool(name="xin", bufs=4) as xpool, \
             tc.tile_pool(name="tmp", bufs=3) as tpool, \
             tc.tile_pool(name="ptp", bufs=8) as ptpool, \
             tc.tile_pool(name="osb", bufs=3) as opool, \
             tc.tile_pool(name="fin", bufs=2) as fpool, \
             tc.tile_pool(name="ps_proj", bufs=2, space="PSUM") as ps_proj, \
             tc.tile_pool(name="ps_s", bufs=2, space="PSUM") as ps_s, \
             tc.tile_pool(name="ps_o", bufs=1, space="PSUM") as ps_o, \
             tc.tile_pool(name="dram", bufs=1, space="DRAM") as dpool:

            wq_sb = cpool.tile([128, 8, 128], BF16)
            wk_sb = cpool.tile([128, 8, 128], BF16)
            wv_sb = cpool.tile([128, 8, 128], BF16)
            wo_sb = cpool.tile([128, 8, HID], BF16)
            nc.sync.dma_start(wq_sb[:], wq_e.ap().rearrange("(kt p) m -> p kt m", p=128))
            nc.sync.dma_start(wk_sb[:], wk_e.ap().rearrange("(kt p) m -> p kt m", p=128))
            nc.sync.dma_start(wv_sb[:], wv_e.ap().rearrange("(kt p) m -> p kt m", p=128))
            cos_sb = cpool.tile([128, L], BF16)
            sin_sb = cpool.tile([128, L], BF16)
            nc.sync.dma_start(cos_sb[:], cos_e[:, :])
            nc.sync.dma_start(sin_sb[:], sin_e[:, :])
            tri2_sb = cpool.tile([128, 2, 128], BF16)
            nc.sync.dma_start(tri2_sb[:, 0], tri_e[:, :])
            nc.sync.dma_start(tri2_sb[:, 1], tri_e[:, :])
            nc.sync.dma_start(
                wo_sb[:], wo_e.ap().rearrange("(kt p) m -> p kt m", p=128))

            # persistent per-core tensors
            qT_sb = ppool.tile([128, B, L], BF16)   # [64*hl+dperm, b, l]
            kT_sb = ppool.tile([128, B, L], BF16)
            # [lk%128, b, lk//128, 65*hl + (d|ones)]; ones column at e=64
            v_sb = ppool.tile([128, B, NT, 130], BF16)

            # AllToAll buffers, one per (chunk, batch):
            # [peer, 128 rows = 64h+d, 64-wide l slice]
            a2a_in = [[dpool.tile([NC, 128, 64], BF16, name=f"a2i{j}{b}",
                                  tag=f"a2i{j}{b}") for b in range(B)]
                      for j in range(NCH)]
            a2a_out = [[dpool.tile([NC, 128, 64], BF16, name=f"a2o{j}{b}",
                                   tag=f"a2o{j}{b}") for b in range(B)]
                       for j in range(NCH)]

            def make_proj(j, b):
                """Prefetch x for chunk (j,b) now; return filler steps that
                emit its QKV projections + RoPE."""
                ls = j * CH
                xc = xpool.tile([128, 8, CH], BF16, name="xc")
                for kt in range(8):
                    nc.sync.dma_start(xc[:, kt], xT_r[b][:, kt, ls:ls + CH])
                steps = []
                st = {}

                def qk_mm(w_sb, dst, kts, key):
                    def go():
                        if kts[0] == 0:
                            st[key] = ps_proj.tile([128, CH], F32, tag="proj",
                                                   name="pp")
                        pp = st[key]
                        for kt in kts:
                            nc.tensor.matmul(pp[:], w_sb[:, kt], xc[:, kt],
                                             start=(kt == 0), stop=(kt == 7),
                                             skip_group_check=True)
                    return go

                def qk_rope(dst, key):
                    def go():
                        pp = st[key]
                        t1 = tpool.tile([128, CH], BF16, name="t1")
                        nc.vector.tensor_mul(t1[:], pp[:], cos_sb[:, ls:ls + CH])
                        sh = tpool.tile([128, CH], F32, name="sh")
                        nc.vector.stream_shuffle(sh[:], pp[:], ROT_MASK)
                        t2 = tpool.tile([128, CH], BF16, name="t2")
                        nc.vector.tensor_mul(t2[:], sh[:], sin_sb[:, ls:ls + CH])
                        nc.vector.tensor_add(dst[:, b, ls:ls + CH], t1[:], t2[:])
                    return go

                for w_sb, dst, key in ((wq_sb, qT_sb, 'q'), (wk_sb, kT_sb, 'k')):
                    for kts in ((0, 1), (2, 3), (4, 5), (6, 7)):
                        steps.append(qk_mm(w_sb, dst, kts, key))
                    steps.append(qk_rope(dst, key))

                def v_mm(tl, kts):
                    t = 4 * j + tl
                    def go():
                        if kts[0] == 0:
                            st['v'] = ps_proj.tile([128, 128], F32, tag="proj",
                                                   name="pv")
                        pv = st['v']
                        for kt in kts:
                            nc.tensor.matmul(pv[:], xc[:, kt, 128 * tl:128 * tl + 128],
                                             wv_sb[:, kt], start=(kt == 0),
                                             stop=(kt == 7), skip_group_check=True)
                        if kts[-1] == 7:
                            nc.vector.memset(v_sb[:, b, t, 64::65], 1.0)
                            vv = v_sb.rearrange("p b t (hl e) -> p b t hl e", e=65)
                            nc.vector.tensor_copy(
                                vv[:, b, t, :, 0:64],
                                pv.rearrange("p (hl d) -> p hl d", d=64))
                    return go

                for tl in range(4):
                    steps.append(v_mm(tl, (0, 1, 2, 3)))
                    steps.append(v_mm(tl, (4, 5, 6, 7)))
                return steps

            def make_wo(jj):
                """Filler steps for chunk jj's output projection (needs both
                A2A(jj,*) done)."""
                st = {}

                def load():
                    oF = fpool.tile([128, 8, B, 64], BF16, name="oF")
                    st['oF'] = oF
                    for b in range(B):
                        eng = nc.sync if b == 0 else nc.scalar
                        eng.dma_start(oF[:, :, b, :],
                                      a2a_out[jj][b].rearrange("p hd l -> hd p l"))

                def mm(nn, ps):
                    def go():
                        if ps[0] == 0:
                            st['py'] = ps_proj.tile([128, 512], F32, tag="proj",
                                                    name="py")
                        py_ = st['py']
                        oF = st['oF']
                        for p in ps:
                            nc.tensor.matmul(py_[:], oF[:, p],
                                             wo_sb[:, p, 512 * nn:512 * nn + 512],
                                             start=(p == 0), stop=(p == 7),
                                             skip_group_check=True)
                        if ps[-1] == 7:
                            ysb = fpool.tile([128, 512], F32, name="ysb")
                            nc.scalar.activation(ysb[:], py_[:], AF.Copy)
                            nc.sync.dma_start(
                                out_e[jj][:, 512 * nn:512 * nn + 512], ysb[:])
                    return go

                steps = [load]
                for nn in range(2):
                    steps.append(mm(nn, (0, 1, 2, 3)))
                    steps.append(mm(nn, (4, 5, 6, 7)))
                return steps

            def attn_chunk(j, b, feeder):
                ls = j * CH
                # po partitions: 0..63 = o dims, 64 = denominator (ones row)
                po = ps_o.tile([65, 2, CH], F32, tag="o", name="po")
                nt_j = 4 * j + 4
                pts = [None] * nt_j
                o0s = [0] * nt_j

                def emit_av(t):
                    for hl in range(2):
                        nc.tensor.matmul(po[:, hl, o0s[t]:CH],
                                         v_sb[:, b, t, 65 * hl:65 * hl + 65],
                                         pts[t][:, hl, o0s[t]:CH],
                                         start=(t == 0), stop=(t == nt_j - 1),
                                         skip_group_check=True)

                for t in range(nt_j):
                    tl = t - 4 * j
                    o0 = 128 * tl if tl > 0 else 0   # skip fully-masked cols
                    o0s[t] = o0
                    pss = ps_s.tile([128, 2, CH], F32, tag="s", name="pss")
                    for hl in range(2):
                        hp = 64 * hl
                        nc.tensor.matmul(
                            pss[:, hl, o0:CH],
                            kT_sb[hp:hp + 64, b, 128 * t:128 * t + 128],
                            qT_sb[hp:hp + 64, b, ls + o0:ls + CH],
                            start=True, stop=True)
                    pt = ptpool.tile([128, 2, CH], BF16, name="pt")
                    nc.scalar.activation(pt[:, :, o0:CH], pss[:, :, o0:CH],
                                         AF.Exp, scale=0.125)
                    pts[t] = pt
                    if tl >= 0:
                        nc.vector.tensor_mul(pt[:, :, o0:o0 + 128],
                                             pt[:, :, o0:o0 + 128], tri2_sb[:])
                    # filler: next chunk's projections / pending Wo, queued
                    # between scores(t) and AV(t-1) so the PE always has work
                    # while the exp of tile t runs on ACT
                    for fn in feeder.take(nt_j - t):
                        fn()
                    if t > 0:
                        emit_av(t - 1)
                emit_av(nt_j - 1)

                # ---- epilogue: divide by denominator, ship to AllToAll ----
                # NB: custom-DVE recip misreads PSUM at base partition 64,
                # so stage the denominator rows through SBUF first.
                dsb = tpool.tile([1, 2, CH], F32, name="dsb", bufs=2)
                nc.vector.tensor_copy(dsb[:], po[64:65, :, :])
                rec = tpool.tile([1, 2, CH], F32, name="rec", bufs=2)
                nc.vector.reciprocal_approx_fast(rec[:], dsb[:])
                rb = tpool.tile([64, 2, CH], F32, name="rb", bufs=2)
                nc.gpsimd.partition_broadcast(rb[:], rec[:])
                o_sb = opool.tile([128, CH], BF16, name="o_sb")
                for hl in range(2):
                    nc.vector.tensor_mul(o_sb[64 * hl:64 * hl + 64, :],
                                         po[0:64, hl, :], rb[:, hl, :])
                # scatter into per-peer pieces: rows (h d), cols = l-slice
                for g in range(2):
                    eng = nc.sync if g == 0 else nc.scalar
                    eng.dma_start(
                        a2a_in[j][b][4 * g:4 * g + 4].rearrange("c hd l -> hd c l"),
                        o_sb[:, 256 * g:256 * g + 256].rearrange(
                            "p (c l) -> p c l", l=64))
                nc.gpsimd.collective_compute(
                    "AllToAll", ALU.bypass,
                    replica_groups=[list(range(NC))],
                    ins=[a2a_in[j][b][:].opt()],
                    outs=[a2a_out[j][b][:].opt()],
                )

            order = [(j, b) for j in range(NCH) for b in range(B)]
            feeder = Feeder()
            for fn in make_proj(0, 0):
                fn()
            for idx, (j, b) in enumerate(order):
                if idx + 1 < len(order):
                    feeder.add(make_proj(*order[idx + 1]))
                if (j, b) == (NCH - 1, 0):
                    feeder.add(make_wo(0))
                if (j, b) == (NCH - 1, 1):
                    feeder.add(make_wo(1))
                    feeder.add(make_wo(2))
                attn_chunk(j, b, feeder)
                for fn in feeder.drain():
                    fn()
            for fn in make_wo(NCH - 1):
                fn()

    nc.compile()
    return nc


def _perm():
    # partition 2k <- dim k, partition 2k+1 <- dim k+32 (per 64-dim head)
    p = np.empty(64, dtype=np.int64)
    k = np.arange(32)
    p[2 * k] = k
    p[2 * k + 1] = k + 32
    return p


def _shards(x, Wq, Wk, Wv, Wo, cos, sin):
    bf = ml_dtypes.bfloat16
    xT = np.ascontiguousarray(x.transpose(0, 2, 1)).astype(bf)          # (B, HID, L)
    woT = np.ascontiguousarray(Wo.T).astype(bf)                          # (HID, HID)
    perm = _perm()

    cosT = cos.T.astype(np.float32)                                      # (D, L)
    sinT = sin.T.astype(np.float32)
    # permuted tables: row 2k -> (cos[k], -sin[k]); row 2k+1 -> (cos[k+32], sin[k+32])
    cosP = cosT[perm]                                                    # (64, L)
    sgn = np.where(np.arange(64) % 2 == 0, -1.0, 1.0)[:, None].astype(np.float32)
    sinP = sinT[perm] * sgn                                              # (64, L)
    cos2 = np.concatenate([cosP, cosP], axis=0).astype(bf)               # (128, L)
    sin2 = np.ascontiguousarray(np.concatenate([sinP, sinP], axis=0)).astype(bf)
    # lower-triangular 128x128 mask for the diagonal band
    p = np.arange(128)[:, None]
    f = np.arange(128)[None, :]
    tri = (p <= f).astype(np.float32).astype(bf)                         # (128, 128)

    hperm = np.concatenate([perm, perm + 64])                            # (128,)

    in_maps = []
    for c in range(NC):
        rows = slice(2 * c * 64, (2 * c + 2) * 64)
        wqT = np.ascontiguousarray(Wq[rows].T[:, hperm]).astype(bf)
        wkT = np.ascontiguousarray(Wk[rows].T[:, hperm]).astype(bf)
        in_maps.append({
            "xT": xT,
            "wq": wqT,
            "wk": wkT,
            "wv": np.ascontiguousarray(Wv[rows].T).astype(bf),
            "wo": woT,
            "cosp": cos2,
            "sinpm": sin2,
            "tri": tri,
        })
    return in_maps


def kernel(x, Wq, Wk, Wv, Wo, cos, sin, trace=False):
    x = np.asarray(x, dtype=np.float32)
    Wq = np.asarray(Wq, dtype=np.float32)
    Wk = np.asarray(Wk, dtype=np.float32)
    Wv = np.asarray(Wv, dtype=np.float32)
    Wo = np.asarray(Wo, dtype=np.float32)
    cos = np.asarray(cos, dtype=np.float32)
    sin = np.asarray(sin, dtype=np.float32)

    if "nc" not in _CACHE:
        _CACHE["nc"] = build()
    nc = _CACHE["nc"]

    in_maps = _shards(x, Wq, Wk, Wv, Wo, cos, sin)
    kw = {}
    if trace and _CACHE.get("trace_all_cores"):
        kw["trace_cores"] = list(range(NC))
    res = bass_utils.run_bass_kernel_spmd(
        nc, in_maps, core_ids=list(range(NC)), trace=trace, **kw)
    _CACHE["last_result"] = res

    y = np.empty((B, L, HID), dtype=np.float32)
    for c in range(NC):
        o = res.results[c]["out"]                     # (NCH, 128, HID)
        for j in range(NCH):
            for b in range(B):
                y[b, CH * j + 64 * c: CH * j + 64 * c + 64, :] = \
                    o[j, 64 * b:64 * b + 64, :]
    return y


if __name__ == "__main__":
    rng = np.random.default_rng(0)
    sc = 1.0 / np.sqrt(HID)
    inputs = {
        "x": rng.standard_normal((B, L, HID), dtype=np.float32),
        "Wq": rng.standard_normal((HID, HID), dtype=np.float32) * sc,
        "Wk": rng.standard_normal((HID, HID), dtype=np.float32) * sc,
        "Wv": rng.standard_normal((HID, HID), dtype=np.float32) * sc,
        "Wo": rng.standard_normal((HID, HID), dtype=np.float32) * sc,
        "cos": rng.random((L, D), dtype=np.float32),
        "sin": rng.random((L, D), dtype=np.float32),
    }
    y = kernel(**inputs)
    print("ran:", y.shape, y.dtype)


# revision 15
# speedup vs baseline: 1.1480x; 1.0372x over previous
"""Distributed RoPE causal attention for Trainium2 (8 NeuronCores).

Problem: B=2, L=2048, H=16 heads, D=64; y = Attn(x) with RoPE on q,k,
causal softmax, then output projection. fp32 I/O.

Sharding: each core owns 2 heads (tensor-parallel over the 16 heads) for
BOTH batches.  Per (chunk, batch) an 8-core AllToAll redistributes the
per-head attention outputs so that core c receives all 16 heads for the
64-wide l-slice [64c, 64c+64) of that chunk; the (replicated-weight)
output projection for each chunk is interleaved into later attention
loops, so only the last small AllToAll is exposed.

The attention t-loop is ACT-(exp)-bound; the next chunk's QKV
projection matmuls and the pending output projections are emitted as
filler steps INSIDE the t-loop so the PE never idles (and stays at max
p-state).

RoPE trick: the q/k head dims are PERMUTED (python-side, in the weight
shards and cos/sin tables) so partition 2k holds dim k and 2k+1 holds
dim k+32.  rotate_half then becomes a single intra-quadrant
stream_shuffle (mask i^1) with the signs folded into the sin table.
Scores are invariant to the permutation (q and k share it).

Matmuls run in bf16 (fp32 PSUM accumulation); softmax plumbing fp32.
"""
import math
import sys

sys.path.insert(0, "/opt/trn_rl_repo")

import numpy as np
import ml_dtypes

import concourse.bass as bass
import concourse.tile as tile
from concourse import bacc, mybir
from concourse import bass_utils

B, L, H, D = 2, 2048, 16, 64
HID = H * D
NC = 8
CH = 512          # lq chunk width
NCH = L // CH     # 4 chunks per batch
NT = L // 128     # 16 k-tiles of 128 per batch
F32 = mybir.dt.float32
BF16 = mybir.dt.bfloat16
AF = mybir.ActivationFunctionType
ALU = mybir.AluOpType

ROT_MASK = [i ^ 1 for i in range(32)]

_CACHE = {}


class Feeder:
    """Queue of emission closures drained evenly across t-loop iterations."""

    def __init__(self):
        self.q = []

    def add(self, steps):
        self.q.extend(steps)

    def take(self, remaining_iters):
        if not self.q:
            return []
        n = math.ceil(len(self.q) / max(remaining_iters, 1))
        out, self.q = self.q[:n], self.q[n:]
        return out

    def drain(self):
        out, self.q = self.q, []
        return out


def build():
    nc = bacc.Bacc("TRN2", target_bir_lowering=False, debug=False, num_devices=NC)

    xT_e = nc.dram_tensor("xT", [B, HID, L], BF16, kind="ExternalInput")
    wq_e = nc.dram_tensor("wq", [HID, 128], BF16, kind="ExternalInput")
    wk_e = nc.dram_tensor("wk", [HID, 128], BF16, kind="ExternalInput")
    wv_e = nc.dram_tensor("wv", [HID, 128], BF16, kind="ExternalInput")
    wo_e = nc.dram_tensor("wo", [HID, HID], BF16, kind="ExternalInput")
    cos_e = nc.dram_tensor("cosp", [128, L], BF16, kind="ExternalInput")
    sin_e = nc.dram_tensor("sinpm", [128, L], BF16, kind="ExternalInput")
    tri_e = nc.dram_tensor("tri", [128, 128], BF16, kind="ExternalInput")
    out_e = nc.dram_tensor("out", [NCH, 128, HID], F32, kind="ExternalOutput")

    xT_r = xT_e.ap().rearrange("b (kt p) l -> b p kt l", p=128)

    with tile.TileContext(nc) as tc:
        with tc.tile_pool(name="const", bufs=1) as cpool, \
             tc.tile_pool(name="persist", bufs=1) as ppool, \
             tc.tile_pool(name="xin", bufs=4) as xpool, \
             tc.tile_pool(name="tmp", bufs=3) as tpool, \
             tc.tile_pool(name="ptp", bufs=8) as ptpool, \
             tc.tile_pool(name="osb", bufs=3) as opool, \
             tc.tile_pool(name="fin", bufs=2) as fpool, \
             tc.tile_pool(name="ps_proj", bufs=2, space="PSUM") as ps_proj, \
             tc.tile_pool(name="ps_s", bufs=2, space="PSUM") as ps_s, \
             tc.tile_pool(name="ps_o", bufs=1, space="PSUM") as ps_o, \
             tc.tile_pool(name="dram", bufs=1, space="DRAM") as dpool:

            wq_sb = cpool.tile([128, 8, 128], BF16)
            wk_sb = cpool.tile([128, 8, 128], BF16)
            wv_sb = cpool.tile([128, 8, 128], BF16)
            wo_sb = cpool.tile([128, 8, HID], BF16)
            nc.sync.dma_start(wq_sb[:], wq_e.ap().rearrange("(kt p) m -> p kt m", p=128))
            nc.sync.dma_start(wk_sb[:], wk_e.ap().rearrange("(kt p) m -> p kt m", p=128))
            nc.sync.dma_start(wv_sb[:], wv_e.ap().rearrange("(kt p) m -> p kt m", p=128))
            cos_sb = cpool.tile([128, L], BF16)
            sin_sb = cpool.tile([128, L], BF16)
            nc.sync.dma_start(cos_sb[:], cos_e[:, :])
            nc.sync.dma_start(sin_sb[:], sin_e[:, :])
            tri2_sb = cpool.tile([128, 2, 128], BF16)
            nc.sync.dma_start(tri2_sb[:, 0], tri_e[:, :])
            nc.sync.dma_start(tri2_sb[:, 1], tri_e[:, :])
            nc.sync.dma_start(
                wo_sb[:], wo_e.ap().rearrange("(kt p) m -> p kt m", p=128))

            # persistent per-core tensors
            qT_sb = ppool.tile([128, B, L], BF16)   # [64*hl+dperm, b, l]
            kT_sb = ppool.tile([128, B, L], BF16)
            # [lk%128, b, lk//128, 65*hl + (d|ones)]; ones column at e=64
            v_sb = ppool.tile([128, B, NT, 130], BF16)

            # AllToAll buffers, one per chunk:
            # [peer, 256 rows = 128b + 64h + d, 64-wide l slice]
            a2a_in = [dpool.tile([NC, 2 * 128, 64], BF16, name=f"a2i{j}",
                                 tag=f"a2i{j}") for j in range(NCH)]
            a2a_out = [dpool.tile([NC, 2 * 128, 64], BF16, name=f"a2o{j}",
                                  tag=f"a2o{j}") for j in range(NCH)]

            def make_proj(j, b):
                """Prefetch x for chunk (j,b) now; return filler steps that
                emit its QKV projections + RoPE."""
                ls = j * CH
                xc = xpool.tile([128, 8, CH], BF16, name="xc")
                for kt in range(8):
                    nc.sync.dma_start(xc[:, kt], xT_r[b][:, kt, ls:ls + CH])
                steps = []
                st = {}

                def qk_mm(w_sb, dst, kts, key):
                    def go():
                        if kts[0] == 0:
                            st[key] = ps_proj.tile([128, CH], F32, tag="proj",
                                                   name="pp")
                        pp = st[key]
                        for kt in kts:
                            nc.tensor.matmul(pp[:], w_sb[:, kt], xc[:, kt],
                                             start=(kt == 0), stop=(kt == 7),
                                             skip_group_check=True)
                    return go

                def qk_rope(dst, key):
                    def go():
                        pp = st[key]
                        t1 = tpool.tile([128, CH], BF16, name="t1")
                        nc.vector.tensor_mul(t1[:], pp[:], cos_sb[:, ls:ls + CH])
                        sh = tpool.tile([128, CH], F32, name="sh")
                        nc.vector.stream_shuffle(sh[:], pp[:], ROT_MASK)
                        t2 = tpool.tile([128, CH], BF16, name="t2")
                        nc.vector.tensor_mul(t2[:], sh[:], sin_sb[:, ls:ls + CH])
                        nc.vector.tensor_add(dst[:, b, ls:ls + CH], t1[:], t2[:])
                    return go

                for w_sb, dst, key in ((wq_sb, qT_sb, 'q'), (wk_sb, kT_sb, 'k')):
                    for kts in ((0, 1), (2, 3), (4, 5), (6, 7)):
                        steps.append(qk_mm(w_sb, dst, kts, key))
                    steps.append(qk_rope(dst, key))

                def v_mm(tl, kts):
                    t = 4 * j + tl
                    def go():
                        if kts[0] == 0:
                            st['v'] = ps_proj.tile([128, 128], F32, tag="proj",
                                                   name="pv")
                        pv = st['v']
                        for kt in kts:
                            nc.tensor.matmul(pv[:], xc[:, kt, 128 * tl:128 * tl + 128],
                                             wv_sb[:, kt], start=(kt == 0),
                                             stop=(kt == 7), skip_group_check=True)
                        if kts[-1] == 7:
                            nc.vector.memset(v_sb[:, b, t, 64::65], 1.0)
                            vv = v_sb.rearrange("p b t (hl e) -> p b t hl e", e=65)
                            nc.vector.tensor_copy(
                                vv[:, b, t, :, 0:64],
                                pv.rearrange("p (hl d) -> p hl d", d=64))
                    return go

                for tl in range(4):
                    steps.append(v_mm(tl, (0, 1, 2, 3)))
                    steps.append(v_mm(tl, (4, 5, 6, 7)))
                return steps

            def make_wo(jj):
                """Filler steps for chunk jj's output projection (needs both
                A2A(jj,*) done)."""
                st = {}

                def load():
                    oF = fpool.tile([128, 8, B, 64], BF16, name="oF")
                    st['oF'] = oF
                    for b in range(B):
                        eng = nc.sync if b == 0 else nc.scalar
                        eng.dma_start(
                            oF[:, :, b, :],
                            a2a_out[jj][:, 128 * b:128 * b + 128, :].rearrange(
                                "p hd l -> hd p l"))

                def mm(nn, ps):
                    def go():
                        if ps[0] == 0:
                            st['py'] = ps_proj.tile([128, 512], F32, tag="proj",
                                                    name="py")
                        py_ = st['py']
                        oF = st['oF']
                        for p in ps:
                            nc.tensor.matmul(py_[:], oF[:, p],
                                             wo_sb[:, p, 512 * nn:512 * nn + 512],
                                             start=(p == 0), stop=(p == 7),
                                             skip_group_check=True)
                        if ps[-1] == 7:
                            ysb = fpool.tile([128, 512], F32, name="ysb")
                            nc.scalar.activation(ysb[:], py_[:], AF.Copy)
                            # split the 256KB store across queues (a single
                            # DMA engine moves only ~22GB/s)
                            for q in range(4):
                                eng = nc.sync if q % 2 == 0 else nc.scalar
                                c0 = 512 * nn + 128 * q
                                eng.dma_start(out_e[jj][:, c0:c0 + 128],
                                              ysb[:, 128 * q:128 * q + 128])
                    return go

                steps = [load]
                for nn in range(2):
                    steps.append(mm(nn, (0, 1, 2, 3)))
                    steps.append(mm(nn, (4, 5, 6, 7)))
                return steps

            def attn_chunk(j, b, feeder):
                ls = j * CH
                # po partitions: 0..63 = o dims, 64 = denominator (ones row)
                po = ps_o.tile([65, 2, CH], F32, tag="o", name="po")
                nt_j = 4 * j + 4
                pts = [None] * nt_j
                o0s = [0] * nt_j

                def emit_av(t):
                    for hl in range(2):
                        nc.tensor.matmul(po[:, hl, o0s[t]:CH],
                                         v_sb[:, b, t, 65 * hl:65 * hl + 65],
                                         pts[t][:, hl, o0s[t]:CH],
                                         start=(t == 0), stop=(t == nt_j - 1),
                                         skip_group_check=True)

                for t in range(nt_j):
                    tl = t - 4 * j
                    o0 = 128 * tl if tl > 0 else 0   # skip fully-masked cols
                    o0s[t] = o0
                    pss = ps_s.tile([128, 2, CH], F32, tag="s", name="pss")
                    for hl in range(2):
                        hp = 64 * hl
                        nc.tensor.matmul(
                            pss[:, hl, o0:CH],
                            kT_sb[hp:hp + 64, b, 128 * t:128 * t + 128],
                            qT_sb[hp:hp + 64, b, ls + o0:ls + CH],
                            start=True, stop=True)
                    pt = ptpool.tile([128, 2, CH], BF16, name="pt")
                    nc.scalar.activation(pt[:, :, o0:CH], pss[:, :, o0:CH],
                                         AF.Exp, scale=0.125)
                    pts[t] = pt
                    if tl >= 0:
                        nc.vector.tensor_mul(pt[:, :, o0:o0 + 128],
                                             pt[:, :, o0:o0 + 128], tri2_sb[:])
                    # filler: next chunk's projections / pending Wo, queued
                    # between scores(t) and AV(t-1) so the PE always has work
                    # while the exp of tile t runs on ACT
                    for fn in feeder.take(nt_j - t):
                        fn()
                    if t > 0:
                        emit_av(t - 1)
                emit_av(nt_j - 1)

                # ---- epilogue: divide by denominator, ship to AllToAll ----
                # NB: custom-DVE recip misreads PSUM at base partition 64,
                # so stage the denominator rows through SBUF first.
                dsb = tpool.tile([1, 2, CH], F32, name="dsb", bufs=2)
                nc.vector.tensor_copy(dsb[:], po[64:65, :, :])
                rec = tpool.tile([1, 2, CH], F32, name="rec", bufs=2)
                nc.vector.reciprocal_approx_fast(rec[:], dsb[:])
                rb = tpool.tile([64, 2, CH], F32, name="rb", bufs=2)
                nc.gpsimd.partition_broadcast(rb[:], rec[:])
                o_sb = opool.tile([128, CH], BF16, name="o_sb")
                for hl in range(2):
                    nc.vector.tensor_mul(o_sb[64 * hl:64 * hl + 64, :],
                                         po[0:64, hl, :], rb[:, hl, :])
                # scatter into per-peer pieces: rows (b h d), cols = l-slice
                for g in range(4):
                    eng = nc.sync if g % 2 == 0 else nc.scalar
                    eng.dma_start(
                        a2a_in[j][2 * g:2 * g + 2,
                                  128 * b:128 * b + 128, :].rearrange(
                            "c hd l -> hd c l"),
                        o_sb[:, 128 * g:128 * g + 128].rearrange(
                            "p (c l) -> p c l", l=64))
                if b == B - 1:
                    nc.gpsimd.collective_compute(
                        "AllToAll", ALU.bypass,
                        replica_groups=[list(range(NC))],
                        ins=[a2a_in[j][:].opt()],
                        outs=[a2a_out[j][:].opt()],
                    )

            order = [(j, b) for j in range(NCH) for b in range(B)]
            feeder = Feeder()
            for fn in make_proj(0, 0):
                fn()
            for idx, (j, b) in enumerate(order):
                if idx + 1 < len(order):
                    feeder.add(make_proj(*order[idx + 1]))
                if (j, b) == (NCH - 1, 0):
                    feeder.add(make_wo(0))
                if (j, b) == (NCH - 1, 1):
                    feeder.add(make_wo(1))
                    feeder.add(make_wo(2))
                attn_chunk(j, b, feeder)
                for fn in feeder.drain():
                    fn()
            for fn in make_wo(NCH - 1):
                fn()

    nc.compile()
    return nc


def _perm():
    # partition 2k <- dim k, partition 2k+1 <- dim k+32 (per 64-dim head)
    p = np.empty(64, dtype=np.int64)
    k = np.arange(32)
    p[2 * k] = k
    p[2 * k + 1] = k + 32
    return p


def _shards(x, Wq, Wk, Wv, Wo, cos, sin):
    bf = ml_dtypes.bfloat16
    xT = np.ascontiguousarray(x.transpose(0, 2, 1)).astype(bf)          # (B, HID, L)
    woT = np.ascontiguousarray(Wo.T).astype(bf)                          # (HID, HID)
    perm = _perm()

    cosT = cos.T.astype(np.float32)                                      # (D, L)
    sinT = sin.T.astype(np.float32)
    # permuted tables: row 2k -> (cos[k], -sin[k]); row 2k+1 -> (cos[k+32], sin[k+32])
    cosP = cosT[perm]                                                    # (64, L)
    sgn = np.where(np.arange(64) % 2 == 0, -1.0, 1.0)[:, None].astype(np.float32)
    sinP = sinT[perm] * sgn                                              # (64, L)
    cos2 = np.concatenate([cosP, cosP], axis=0).astype(bf)               # (128, L)
    sin2 = np.ascontiguousarray(np.concatenate([sinP, sinP], axis=0)).astype(bf)
    # lower-triangular 128x128 mask for the diagonal band
    p = np.arange(128)[:, None]
    f = np.arange(128)[None, :]
    tri = (p <= f).astype(np.float32).astype(bf)                         # (128, 128)

    hperm = np.concatenate([perm, perm + 64])                            # (128,)

    in_maps = []
    for c in range(NC):
        rows = slice(2 * c * 64, (2 * c + 2) * 64)
        wqT = np.ascontiguousarray(Wq[rows].T[:, hperm]).astype(bf)
        wkT = np.ascontiguousarray(Wk[rows].T[:, hperm]).astype(bf)
        in_maps.append({
            "xT": xT,
            "wq": wqT,
            "wk": wkT,
            "wv": np.ascontiguousarray(Wv[rows].T).astype(bf),
            "wo": woT,
            "cosp": cos2,
            "sinpm": sin2,
            "tri": tri,
        })
    return in_maps


def kernel(x, Wq, Wk, Wv, Wo, cos, sin, trace=False):
    x = np.asarray(x, dtype=np.float32)
    Wq = np.asarray(Wq, dtype=np.float32)
    Wk = np.asarray(Wk, dtype=np.float32)
    Wv = np.asarray(Wv, dtype=np.float32)
    Wo = np.asarray(Wo, dtype=np.float32)
    cos = np.asarray(cos, dtype=np.float32)
    sin = np.asarray(sin, dtype=np.float32)

    if "nc" not in _CACHE:
        _CACHE["nc"] = build()
    nc = _CACHE["nc"]

    in_maps = _shards(x, Wq, Wk, Wv, Wo, cos, sin)
    kw = {}
    if trace and _CACHE.get("trace_all_cores"):
        kw["trace_cores"] = list(range(NC))
    res = bass_utils.run_bass_kernel_spmd(
        nc, in_maps, core_ids=list(range(NC)), trace=trace, **kw)
    _CACHE["last_result"] = res

    y = np.empty((B, L, HID), dtype=np.float32)
    for c in range(NC):
        o = res.results[c]["out"]                     # (NCH, 128, HID)
        for j in range(NCH):
            for b in range(B):
                y[b, CH * j + 64 * c: CH * j + 64 * c + 64, :] = \
                    o[j, 64 * b:64 * b + 64, :]
    return y


if __name__ == "__main__":
    rng = np.random.default_rng(0)
    sc = 1.0 / np.sqrt(HID)
    inputs = {
        "x": rng.standard_normal((B, L, HID), dtype=np.float32),
        "Wq": rng.standard_normal((HID, HID), dtype=np.float32) * sc,
        "Wk": rng.standard_normal((HID, HID), dtype=np.float32) * sc,
        "Wv": rng.standard_normal((HID, HID), dtype=np.float32) * sc,
        "Wo": rng.standard_normal((HID, HID), dtype=np.float32) * sc,
        "cos": rng.random((L, D), dtype=np.float32),
        "sin": rng.random((L, D), dtype=np.float32),
    }
    y = kernel(**inputs)
    print("ran:", y.shape, y.dtype)
